# revision 1
# baseline (speedup 1.0000x reference)
"""MVS plane-sweep cost-volume kernel for Trainium2 (Bass/Tile), 8 NeuronCores.

v4-NT: non-transpose SWDGE gathers with 512B pair descriptors.
  - DRAM pair table: row = [Z4f16(b) | Z4f16(b+1)] (512B), dual parity views
    so any base b is reachable at slot 0; ref pair table appended
    (row i = [ref4(2i) | ref4(2i+1)]).
  - Pixel pairs (2i, 2i+1): one descriptor covers both when
    q(2i+1) == q(2i)+1 (~94%).  Bad pairs zero-weight the odd pixel; a
    globally-packed "solo" stream re-gathers those pixels (src pair with
    base at slot 1) together with their ref rows (gathered, since solo
    slots are not pixel-aligned).
  - Compute (per 2048-px chunk, pixel-major): DVE f16 multiply by ref
    (2x rate), tensor_reduce over channels, corner-weight multiply,
    tensor_reduce over corners.  Host scatters solo outputs, normalizes.
"""
import numpy as np

B, V, C, H, W = 2, 5, 32, 128, 160
D = 48
HW = H * W
NCORES = 8
PAD = W + 1
NZ = HW + W + 1                 # 20641 base cells
NE = NZ + 3                     # src pair-table rows: row r = [Z4(r-2), Z4(r-1)]
NR = HW // 2                    # ref pair-table rows
SC = 2048                       # pixels per good gather call (1024 pairs)
NSC = HW // SC                  # 10
NSOLO_CH = 42                   # solo chunks of 1024 units (padded, worst ~41k)
NSOLO = NSOLO_CH * 1024

_PROGRAM_CACHE = {}


def _fold(proj):
    out = proj[0].copy()
    out[:3, :4] = (proj[1][:3, :3] @ proj[0][:3, :4]).astype(np.float32)
    return out


def _host_fields(features, proj_matrices, depth_values, view_weights):
    ys, xs = np.meshgrid(np.arange(H, dtype=np.float32),
                         np.arange(W, dtype=np.float32), indexing='ij')
    grid = np.stack([xs.ravel(), ys.ravel(), np.ones(HW, dtype=np.float32)], 0)
    cores = []
    for b in range(B):
        ref_p = _fold(proj_matrices[b, 0])
        ref_p_inv = np.linalg.inv(ref_p.astype(np.float64)).astype(np.float32)
        for v in range(1, V):
            proj = (_fold(proj_matrices[b, v]).astype(np.float64)
                    @ ref_p_inv.astype(np.float64)).astype(np.float32)
            rot, trans = proj[:3, :3], proj[:3, 3]
            rot_xyz = rot.astype(np.float32) @ grid
            dep = depth_values[b].astype(np.float32)
            pxyz = (rot_xyz[:, None, :] * dep[None, :, None]
                    + trans[:, None, None]).astype(np.float32)
            px = (pxyz[0] / pxyz[2]).astype(np.float32)
            py = (pxyz[1] / pxyz[2]).astype(np.float32)
            x0 = np.floor(px)
            y0 = np.floor(py)
            wx = px - x0
            wy = py - y0
            vw = view_weights[b, v - 1].reshape(HW)
            x0c = np.clip(x0, -1, W - 1)
            y0c = np.clip(y0, -1, H - 1)
            q = (y0c * W + x0c + PAD).astype(np.int64)
            wt4 = np.empty((4, D, HW), dtype=np.float32)
            corners = [(x0, y0, (1 - wx) * (1 - wy)),
                       (x0 + 1, y0, wx * (1 - wy)),
                       (x0, y0 + 1, (1 - wx) * wy),
                       (x0 + 1, y0 + 1, wx * wy)]
            for k, (xi, yi, wk) in enumerate(corners):
                valid = ((xi >= 0) & (xi <= W - 1) & (yi >= 0) & (yi <= H - 1))
                wt4[k] = (wk * valid).astype(np.float32) * vw[None, :] / np.float32(C)
            cores.append((b, v, q, wt4))
    return cores


def _e_src(b):
    """base b (array, in [-2, NZ]) -> sliding-window pair-table row (slot0=b)."""
    return (b + 2).astype(np.int64)


def _build_tables(src, ref):
    """-> combined pair table [NE + NR, 256] f16; row = 2 x (s,t,c) packs."""
    npos = NE + 2
    bb = np.arange(npos) - 2
    f4 = np.empty((128, npos), dtype=np.float16)
    r4 = np.empty((128, HW), dtype=np.float16)
    for s in (0, 1):
        for t in (0, 1):
            rows = slice(s * 64 + t * 32, s * 64 + t * 32 + 32)
            f4[rows] = src[:, np.clip(bb - PAD + s * W + t, 0, HW - 1)]
            r4[rows] = ref
    tbl = np.empty((NE + NR, 2, 128), dtype=np.float16)
    rr = np.arange(NE)
    tbl[:NE, 0, :] = f4[:, rr].T             # slot0 = base r-2 (pos r)
    tbl[:NE, 1, :] = f4[:, rr + 1].T         # slot1 = base r-1
    tbl[NE:, 0, :] = r4[:, 0::2].T
    tbl[NE:, 1, :] = r4[:, 1::2].T
    return tbl.reshape(NE + NR, 256)


def _wrap16(idx2d):
    *lead, n = idx2d.shape
    w = idx2d.reshape(*lead, n // 16, 16)
    w = np.swapaxes(w, -1, -2)
    return np.tile(w, (*([1] * len(lead)), 8, 1)).astype(np.int16)


def _pack_core_inputs(features, cores):
    in_maps = []
    for (b, v, q, wt4) in cores:
        src = features[b, v].reshape(C, HW).astype(np.float32)
        ref = features[b, 0].reshape(C, HW).astype(np.float32)
        tbl = _build_tables(src, ref)

        qa = q[:, 0::2]
        qb = q[:, 1::2]
        good = (qb == qa + 1)

        idx_t = _wrap16(_e_src(qa).reshape(D, NSC, SC // 2))

        # ref stream per sc: rr[sc, p, slab, v] = ref4[v%128, pixel]
        # pixel(sc, slab, p, px) = sc*2048 + slab*256 + 2p + px, px = v//128
        r4 = np.tile(ref, (4, 1)).astype(np.float16)          # (128, HW)
        rr = (r4.reshape(128, NSC, 8, 128, 2)                 # p? no: c,sc,slab,pair,px
              )
        # build via pixel indexing: easier with take
        pix = (np.arange(NSC)[:, None, None, None] * 2048
               + np.arange(8)[None, :, None, None] * 256
               + np.arange(128)[None, None, :, None] * 2
               + np.arange(2)[None, None, None, :])           # (NSC,8,128,2)
        rr = r4[:, pix]                                       # (128,NSC,8,128,2)
        rr = rr.transpose(1, 3, 2, 4, 0).reshape(NSC, 128, 8, 256)

        # weights with bad-odd-pixels zeroed; layout [D,NSC,128,(slab,px,k)]
        wt4z = wt4.copy()
        wt4z[:, :, 1::2] *= good[None, :, :]
        wtt = (wt4z.astype(np.float16)
               .reshape(4, D, NSC, 8, 128, 2)
               .transpose(1, 2, 4, 3, 5, 0)                   # d,sc,p,slab,px,k
               .reshape(D, NSC, 128, 64))

        # global solo stream, sorted by base for gather locality
        dd, ii = np.nonzero(~good)
        bq = qb[~good]                                        # bases of bad odd pixels
        order = np.argsort(bq, kind="stable")
        dd, ii, bq = dd[order], ii[order], bq[order]
        nsolo = len(dd)
        assert nsolo <= NSOLO, nsolo
        sidx = np.zeros(NSOLO, dtype=np.int64)
        sidx[:nsolo] = _e_src(bq - 1)
        ws = np.zeros((NSOLO, 4), dtype=np.float16)
        ws[:nsolo] = wt4[:, dd, 2 * ii + 1].T.astype(np.float16)
        # dense host-side ref stream for solo units (both slots = ref4 of B)
        r4f = np.tile(ref, (4, 1)).astype(np.float16)         # (128, HW)
        r1 = np.zeros((NSOLO, 128), dtype=np.float16)
        r1[:nsolo] = r4f[:, 2 * ii + 1].T
        rsolo = (np.concatenate([r1, r1], axis=1)             # (NSOLO, 256)
                 .reshape(NSOLO_CH, 8, 128, 256)              # ch, slab, p, v
                 .transpose(0, 2, 1, 3))                      # ch, p, slab, v
        # unit u -> chunk u//1024, p=(u%1024)%128, slab=(u%1024)//128, px=1
        wsolo = np.zeros((NSOLO_CH, 128, 8, 2, 4), dtype=np.float16)
        u = np.arange(NSOLO)
        wsolo[u // 1024, (u % 1024) % 128, (u % 1024) // 128, 1, :] = ws
        wsolo = wsolo.reshape(NSOLO_CH, 128, 64)

        in_maps.append({
            "tbl": np.ascontiguousarray(tbl),
            "rr": np.ascontiguousarray(rr),
            "idx": np.ascontiguousarray(idx_t),
            "wtt": np.ascontiguousarray(wtt),
            "sidx": np.ascontiguousarray(_wrap16(sidx.reshape(NSOLO_CH, 1024))),
            "rsolo": np.ascontiguousarray(rsolo),
            "wsolo": np.ascontiguousarray(wsolo),
            "_scatter": (dd, 2 * ii + 1),
        })
    return in_maps


def _build_program():
    import concourse.bacc as bacc
    import concourse.tile as tile
    import concourse.mybir as mybir

    nc = bacc.Bacc("TRN2", target_bir_lowering=False, debug=False,
                   num_devices=NCORES, num_swdge_queues=4)
    f32 = mybir.dt.float32
    f16 = mybir.dt.float16
    i16 = mybir.dt.int16
    X = mybir.AxisListType.X
    ADD = mybir.AluOpType.add

    tbl_d = nc.dram_tensor("tbl", [NE + NR, 256], f16, kind="ExternalInput")
    rr_d = nc.dram_tensor("rr", [NSC, 128, 8, 256], f16, kind="ExternalInput")
    idx_d = nc.dram_tensor("idx", [D, NSC, 128, 64], i16, kind="ExternalInput")
    wtt_d = nc.dram_tensor("wtt", [D, NSC, 128, 64], f16, kind="ExternalInput")
    sidx_d = nc.dram_tensor("sidx", [NSOLO_CH, 128, 64], i16, kind="ExternalInput")
    rsolo_d = nc.dram_tensor("rsolo", [NSOLO_CH, 128, 8, 256], f16,
                             kind="ExternalInput")
    wsolo_d = nc.dram_tensor("wsolo", [NSOLO_CH, 128, 64], f16, kind="ExternalInput")
    outg_d = nc.dram_tensor("outg", [D, NSC, 128, 16], f32, kind="ExternalOutput")
    outs_d = nc.dram_tensor("outs", [NSOLO_CH, 128, 16], f32, kind="ExternalOutput")

    with tile.TileContext(nc) as tc:
        with (
            tc.tile_pool(name="rtp", bufs=2) as rtp,
            tc.tile_pool(name="gat", bufs=6) as gat,
            tc.tile_pool(name="grp", bufs=4) as grp,
            tc.tile_pool(name="idxp", bufs=8) as idxp,
            tc.tile_pool(name="c1p", bufs=4) as c1p,
            tc.tile_pool(name="wtp", bufs=4) as wtp,
            tc.tile_pool(name="obp", bufs=4) as obp,
            tc.tile_pool(name="sgat", bufs=6) as sgat,
            tc.tile_pool(name="sidxp", bufs=8) as sidxp,
        ):
            gq = 0
            for sc in range(NSC):
                rt = rtp.tile([128, 2048], f16)
                nc.sync.dma_start(
                    rt[:].rearrange("p (s v) -> p s v", s=8), rr_d.ap()[sc])
                for d in range(D):
                    idxt = idxp.tile([128, 64], i16)
                    nc.sync.dma_start(idxt[:], idx_d.ap()[d, sc])
                    g = gat.tile([128, 2048], f16)
                    nc.gpsimd.dma_gather(
                        g[:].rearrange("p (s v) -> p s v", s=8),
                        tbl_d.ap(), idxt[:],
                        num_idxs=1024, num_idxs_reg=1024, elem_size=256,
                        queue_num=gq % 4)
                    gq += 1
                    gr = grp.tile([128, 2048], f16)
                    nc.vector.tensor_mul(gr[:], g[:], rt[:])
                    c1 = c1p.tile([128, 64], f32)
                    nc.vector.tensor_reduce(
                        c1[:], gr[:].rearrange("p (a c) -> p a c", c=32),
                        axis=X, op=ADD)
                    wt = wtp.tile([128, 64], f16)
                    nc.sync.dma_start(wt[:], wtt_d.ap()[d, sc])
                    nc.vector.tensor_mul(c1[:], c1[:], wt[:])
                    ob = obp.tile([128, 16], f32)
                    nc.vector.tensor_reduce(
                        ob[:], c1[:].rearrange("p (a k) -> p a k", k=4),
                        axis=X, op=ADD)
                    nc.sync.dma_start(outg_d.ap()[d, sc], ob[:])

            for u in range(NSOLO_CH):
                sit = sidxp.tile([128, 64], i16, name="sit")
                nc.sync.dma_start(sit[:], sidx_d.ap()[u])
                gs = sgat.tile([128, 2048], f16, name="gs")
                nc.gpsimd.dma_gather(
                    gs[:].rearrange("p (s v) -> p s v", s=8),
                    tbl_d.ap(), sit[:],
                    num_idxs=1024, num_idxs_reg=1024, elem_size=256,
                    queue_num=gq % 4)
                gq += 1
                gsr = sgat.tile([128, 2048], f16, name="gsr")
                nc.sync.dma_start(
                    gsr[:].rearrange("p (s v) -> p s v", s=8), rsolo_d.ap()[u])
                grs = grp.tile([128, 2048], f16)
                nc.vector.tensor_mul(grs[:], gs[:], gsr[:])
                c1s = c1p.tile([128, 64], f32)
                nc.vector.tensor_reduce(
                    c1s[:], grs[:].rearrange("p (a c) -> p a c", c=32),
                    axis=X, op=ADD)
                wts = wtp.tile([128, 64], f16, name="wts")
                nc.sync.dma_start(wts[:], wsolo_d.ap()[u])
                nc.vector.tensor_mul(c1s[:], c1s[:], wts[:])
                obs = obp.tile([128, 16], f32, name="obs")
                nc.vector.tensor_reduce(
                    obs[:], c1s[:].rearrange("p (a k) -> p a k", k=4),
                    axis=X, op=ADD)
                nc.sync.dma_start(outs_d.ap()[u], obs[:])

    nc.compile()
    return nc


def _get_program():
    if "nc" not in _PROGRAM_CACHE:
        _PROGRAM_CACHE["nc"] = _build_program()
    return _PROGRAM_CACHE["nc"]


def _run(inputs, trace=False):
    from concourse.bass_utils import run_bass_kernel_spmd

    features = np.asarray(inputs["features"], dtype=np.float32)
    proj_matrices = np.asarray(inputs["proj_matrices"], dtype=np.float32)
    depth_values = np.asarray(inputs["depth_values"], dtype=np.float32)
    view_weights = np.asarray(inputs["view_weights"], dtype=np.float32)

    cores = _host_fields(features, proj_matrices, depth_values, view_weights)
    in_maps = _pack_core_inputs(features, cores)
    scatters = [m.pop("_scatter") for m in in_maps]
    nc = _get_program()
    res = run_bass_kernel_spmd(nc, in_maps, core_ids=list(range(NCORES)),
                               trace=trace)

    out = np.empty((B, 1, D, H, W), dtype=np.float32)
    for b in range(B):
        vol = np.zeros((D, HW), dtype=np.float32)
        wsum = np.full((HW,), 1e-5, dtype=np.float32)
        for v in range(1, V):
            ci = b * 4 + (v - 1)
            # outg [D, NSC, 128, 16]: pixel = sc*2048 + slab*256 + 2p + px
            og = (res.results[ci]["outg"].reshape(D, NSC, 128, 8, 2)
                  .transpose(0, 1, 3, 2, 4).reshape(D, HW))
            osv = res.results[ci]["outs"].reshape(NSOLO_CH, 128, 8, 2)
            dd, bpix = scatters[ci]
            u = np.arange(len(dd))
            og[dd, bpix] = osv[u // 1024, (u % 1024) % 128, (u % 1024) // 128, 1]
            vol = vol + og
            wsum = wsum + view_weights[b, v - 1].reshape(HW)
        out[b, 0] = (vol / wsum[None, :]).reshape(D, H, W)
    return out, res


def kernel(**inputs) -> np.ndarray:
    out, _ = _run(inputs, trace=False)
    return out

